# revision 7
# baseline (speedup 1.0000x reference)
"""FakeQuantLinear Trainium2 kernel (8-core data-parallel over tokens).

Math (per reference):
    x_int8 = clip(round(x / scale_a) + zp_a, -128, 127)
    y = (x_int8 - zp_a) @ (W - w_zp).T * (scale_a * w_scale) + bias

Key facts exploited:
  * (x_int8 - zp_a) and (W - w_zp) are small integers (|v| <= ~131), exactly
    representable in fp16 -> the TensorE fp16 matmul with f32 PSUM
    accumulation is (near-)exact, and fp16 matmul runs at the same
    1 cycle/row as bf16 on TRN2.
  * Quantization in TWO device ops via an fp16 magic shift: every integer in
    [1024, 2048) is exactly representable in fp16 with ulp 1, so converting
    f32(x*inv_s + 1536) to fp16 rounds to the integer grid (RNE, matching
    jnp.round).  The +1536 never gets subtracted on device: the drain's bias
    absorbs -total_scale*1536*rowsum(W) per output feature (computed on host
    in f64).  PSUM partial sums stay integer-exact (< 2^24 up to harmless
    tail rounding that cancels to ~1e-5 relative after the bias fold).
  * clip(round(v) + zp, -128, 127) - zp == clip(round(v), -128-zp, 127-zp),
    so zp folds into the fp16 clamp immediates.

Sharding: data-parallel over the 8192 tokens; each of the 8 cores handles
1024 tokens and holds the full (host-pre-centered, fp16) weight.

Device schedule (per core), PE-floor oriented (32*32*1024 rows / 2.4 GHz
= 437 us):
  * quant: per k-tile, ACT produces t1 = fp16(x*inv_s + 1536) and DVE
    clamps -> xq (fp16, 2x DVE rate).  ~1.4 us/tile; x DMA (DVE queue)
    streams at ~1.4 us/tile.
  * group 0: G=4 o-tiles k-interleaved so PE consumes each xq tile every
    ~1.7 us -- slower than quant+DMA produce them -> no PE starvation
    during the quant ramp.
  * o-tiles G..ot-1: sequential per-o k-sweeps, 2 PSUM banks per o-tile
    (4-deep reuse lookahead), ScalarE drain fused *scale+bias, y DMA on SP.
  * DMA issue queues: x/y/bias on SP, w on gpsimd (Pool) -- independent
    sequencers, neither blocking quant (ACT/DVE) or drain (ACT) work.
"""

import sys

for _p in ("/opt/trn_rl_repo",):
    if _p not in sys.path:
        sys.path.insert(0, _p)

import numpy as np

B, S, IN, OUT = 4, 2048, 4096, 4096
M = B * S  # 8192 tokens
NCORES = 8
MS = M // NCORES  # 1024 tokens per core
MAGIC16 = 1536.0  # 1.5 * 2**10: fp16 round-to-nearest-even shifter


def build_bass(ms, in_dim, out_dim, inv_s, lo, hi, total_scale,
               G=4, psum_bufs=8, w_bufs=None, w_prefetch=10, fast_start=2,
               seq_from=None, loop_n=1, w_dma="gpsimd", x_dma="sync"):
    """Build the per-core Bass/Tile program.

    ms: tokens on this core; in_dim/out_dim: contraction / output features.
    inv_s, lo, hi, total_scale: compile-time immediates from the runtime
    quantization scalars.  seq_from: first o-group index run in sequential
    per-o mode (default 1 = everything after group 0).
    """
    import concourse.bass as bass  # noqa: F401 (side-effect imports)
    import concourse.mybir as mybir
    import concourse.tile as tile
    from concourse import bacc

    kt = in_dim // 128  # k tiles
    ot = out_dim // 128  # o tiles
    mh = ms // 2  # m half (one PSUM bank's worth, <= 512)
    assert mh <= 512
    assert ot % G == 0
    if w_bufs is None:
        w_bufs = 2 * G + 2
    if seq_from is None:
        seq_from = 1

    f32 = mybir.dt.float32
    f16 = mybir.dt.float16
    Act = mybir.ActivationFunctionType
    Alu = mybir.AluOpType

    nc = bacc.Bacc()
    xT_d = nc.dram_tensor("xT", [in_dim, ms], f32, kind="ExternalInput")
    wp_d = nc.dram_tensor("wp", [ot, 128, in_dim], f16, kind="ExternalInput")
    bias_d = nc.dram_tensor("biasc", [128, ot], f32, kind="ExternalInput")
    yT_d = nc.dram_tensor("yT", [out_dim, ms], f32, kind="ExternalOutput")

    with tile.TileContext(nc) as tc:
        with (
            tc.tile_pool(name="xin", bufs=4) as xin_pool,
            tc.tile_pool(name="tmp", bufs=3) as tmp_pool,
            tc.tile_pool(name="xq", bufs=1) as xq_pool,
            tc.tile_pool(name="wts", bufs=w_bufs) as w_pool,
            tc.tile_pool(name="psum", bufs=psum_bufs, space="PSUM") as psum_pool,
            tc.tile_pool(name="yout", bufs=3) as y_pool,
            tc.tile_pool(name="const", bufs=1) as const_pool,
        ):

            def emit_body():
                bias_sb = const_pool.tile([128, ot], f32, tag="bias", name="bias_sb")
                nc.sync.dma_start(out=bias_sb[:], in_=bias_d[:])
                magic_sb = const_pool.tile([128, 1], f32, tag="magic", name="magic_sb")
                nc.vector.memset(magic_sb[:], MAGIC16)

                w_eng = getattr(nc, w_dma)
                x_eng = getattr(nc, x_dma)
                wb_pre = {}

                # Group 0's weights, chunk-DMA'd round-robin so every o-tile's
                # leading k-slices land before PE's first k-sweep reaches them.
                g0 = list(range(min(G, ot)))
                for j in g0:
                    wb_pre[j] = w_pool.tile(
                        [128, in_dim], f16, tag="wb", name=f"wbp_{j}"
                    )
                chunks = 4
                cw = in_dim // chunks
                for c in range(chunks):
                    for j in g0:
                        w_eng.dma_start(
                            out=wb_pre[j][:, c * cw : (c + 1) * cw],
                            in_=wp_d[j][:, c * cw : (c + 1) * cw],
                        )

                # x DMAs hoisted: the x queue streams ahead of quant, bounded
                # by the xin pool depth.
                xf_tiles = []
                for k in range(kt):
                    xf = xin_pool.tile([128, ms], f32, tag="xf", name=f"xf_{k}")
                    splits = [(0, mh), (mh, ms)] if k < fast_start else [(0, ms)]
                    for c0, c1 in splits:
                        x_eng.dma_start(
                            out=xf[:, c0:c1],
                            in_=xT_d[k * 128 : (k + 1) * 128, c0:c1],
                        )
                    xf_tiles.append(xf)

                # Phase 1: quantize x -> fp16 magic-shifted integers, SBUF-
                # resident.  ACT rounds via the fp16 convert; DVE clamps.
                xq_tiles = []
                for k in range(kt):
                    if k % 2 == 1 and G + k // 2 < min(w_prefetch, ot):
                        j = G + k // 2
                        wb = w_pool.tile([128, in_dim], f16, tag="wb",
                                         name=f"wbp_{j}")
                        w_eng.dma_start(out=wb[:], in_=wp_d[j])
                        wb_pre[j] = wb
                    xf = xf_tiles[k]
                    t1 = tmp_pool.tile([128, ms], f16, tag="t1")
                    xq = xq_pool.tile([128, ms], f16, tag=f"xq{k}")
                    splits = [(0, mh), (mh, ms)] if k < fast_start else [(0, ms)]
                    for c0, c1 in splits:
                        s = slice(c0, c1)
                        nc.scalar.activation(
                            t1[:, s], xf[:, s], Act.Identity,
                            bias=magic_sb[:, 0:1], scale=inv_s,
                        )
                        nc.vector.tensor_scalar(
                            out=xq[:, s], in0=t1[:, s],
                            scalar1=MAGIC16 + float(hi),
                            scalar2=MAGIC16 + float(lo),
                            op0=Alu.min, op1=Alu.max,
                        )
                    xq_tiles.append(xq)

                def get_w(j):
                    if j in wb_pre:
                        return wb_pre.pop(j)
                    wb = w_pool.tile([128, in_dim], f16, tag="wb", name=f"wb_{j}")
                    w_eng.dma_start(out=wb[:], in_=wp_d[j])
                    return wb

                def drain(y, ps, j, half):
                    hs = slice(half * mh, (half + 1) * mh)
                    nc.scalar.activation(
                        y[:, hs], ps[:], Act.Identity,
                        bias=bias_sb[:, j : j + 1], scale=total_scale,
                    )
                    nc.sync.dma_start(
                        out=yT_d[j * 128 : (j + 1) * 128, hs], in_=y[:, hs]
                    )

                # Phase 2a: group 0, G o-tiles k-interleaved (each xq[k] feeds
                # 2*G matmuls back-to-back: hides the quant/x-DMA ramp).
                for jg in range(min(seq_from, ot // G)):
                    js = [jg * G + i for i in range(G)]
                    wbs = [get_w(j) for j in js]
                    pss = [
                        (
                            psum_pool.tile([128, mh], f32, tag="ps", name=f"ps0_{j}"),
                            psum_pool.tile([128, mh], f32, tag="ps", name=f"ps1_{j}"),
                        )
                        for j in js
                    ]
                    for k in range(kt):
                        xq0 = xq_tiles[k][:, 0:mh]
                        xq1 = xq_tiles[k][:, mh:ms]
                        st, sp = (k == 0), (k == kt - 1)
                        for i in range(G):
                            lhs = wbs[i][:, k * 128 : (k + 1) * 128]
                            nc.tensor.matmul(pss[i][0][:], lhs, xq0, start=st, stop=sp)
                            nc.tensor.matmul(pss[i][1][:], lhs, xq1, start=st, stop=sp)
                    for i, j in enumerate(js):
                        y = y_pool.tile([128, ms], f32, tag="y", name=f"y_{j}")
                        for half in (0, 1):
                            drain(y, pss[i][half], j, half)

                # Phase 2b: remaining o-tiles sequentially -- each o runs its
                # full k-loop per PSUM half, so drains + y stores overlap the
                # next o-tile's matmuls and PSUM banks cycle 4 deep.
                for j in range(min(seq_from, ot // G) * G, ot):
                    wb = get_w(j)
                    y = y_pool.tile([128, ms], f32, tag="y", name=f"yt_{j}")
                    for half in (0, 1):
                        ps = psum_pool.tile([128, mh], f32, tag="ps",
                                            name=f"ps{half}_{j}")
                        for k in range(kt):
                            lhs = wb[:, k * 128 : (k + 1) * 128]
                            rhs = xq_tiles[k][:, half * mh : (half + 1) * mh]
                            nc.tensor.matmul(
                                ps[:], lhs, rhs,
                                start=(k == 0), stop=(k == kt - 1),
                            )
                        drain(y, ps, j, half)

            if loop_n > 1:
                with tc.For_i(0, loop_n, 1):
                    emit_body()
            else:
                emit_body()

    nc.compile()
    return nc


def prep_inputs(x, weight_int, bias, scale_a, zp_a, weight_scale, weight_zero_point):
    """Host-side layout prep + immediates. Returns (in_maps, immediates)."""
    s_a = float(np.float64(np.asarray(scale_a)))
    zp = float(int(np.asarray(zp_a)))
    s_w = float(np.float64(np.asarray(weight_scale)))
    w_zp = int(np.asarray(weight_zero_point))

    inv_s = float(np.float32(1.0 / np.float64(s_a)))
    lo = -128.0 - zp
    hi = 127.0 - zp
    total_scale = float(np.float32(np.float32(s_a) * np.float32(s_w)))

    m, in_dim = x.reshape(-1, x.shape[-1]).shape
    out_dim = weight_int.shape[0]
    ms = m // NCORES
    ot = out_dim // 128

    X = np.ascontiguousarray(x.reshape(m, in_dim).T.astype(np.float32, copy=False))

    # w_prep[j, p, k*128+c] = Wc[j*128+c, k*128+p]; fp16 host-centered
    # (integers <= ~131: exact).
    w_cent = weight_int.astype(np.int32) - w_zp
    w_prep = np.ascontiguousarray(
        w_cent.astype(np.float16)
        .reshape(ot, 128, in_dim // 128, 128)
        .transpose(0, 3, 2, 1)
    ).reshape(ot, 128, in_dim)

    # Fold the fp16 magic shift out of the GEMM:
    #   psum_j = sum_k Wc[j,k] * (x_int[k] + 1536)
    #   y_j    = total_scale * psum_j + (bias_j - total_scale*1536*rowsum_j)
    rowsum = w_cent.astype(np.float64).sum(axis=1)
    bias_fold = bias.astype(np.float64) - float(total_scale) * MAGIC16 * rowsum
    bias_col = np.ascontiguousarray(
        bias_fold.astype(np.float32).reshape(ot, 128).T
    )

    in_maps = []
    for c in range(NCORES):
        in_maps.append(
            {
                "xT": np.ascontiguousarray(X[:, c * ms : (c + 1) * ms]),
                "wp": w_prep,
                "biasc": bias_col,
            }
        )
    return in_maps, (ms, in_dim, out_dim, inv_s, lo, hi, total_scale)


def assemble_output(results, m, out_dim):
    """Concatenate per-core yT shards [OUT, ms] -> y [B, S, OUT]."""
    ys = [np.asarray(r["yT"]).T for r in results]  # each [ms, OUT]
    Y = np.concatenate(ys, axis=0)
    return np.ascontiguousarray(Y.reshape(B, S, out_dim).astype(np.float32))


def run(inputs, trace=False, **spmd_kwargs):
    """Full pipeline returning (y, BassKernelResults). Used by test harness."""
    from concourse.bass_utils import run_bass_kernel_spmd

    in_maps, imm = prep_inputs(**inputs)
    nc = build_bass(*imm)
    res = run_bass_kernel_spmd(
        nc, in_maps, list(range(NCORES)), trace=trace, **spmd_kwargs
    )
    return assemble_output(res.results, M, OUT), res


def kernel(x, weight_int, bias, scale_a, zp_a, weight_scale, weight_zero_point):
    from concourse.bass_utils import run_bass_kernel_spmd

    in_maps, imm = prep_inputs(
        x, weight_int, bias, scale_a, zp_a, weight_scale, weight_zero_point
    )
    nc = build_bass(*imm)
    res = run_bass_kernel_spmd(nc, in_maps, list(range(NCORES)))
    return assemble_output(res.results, M, OUT)
